# revision 15
# baseline (speedup 1.0000x reference)
"""BandSplitLinear Trainium2 kernel (v4: host-transposed fp16 I/O, pure matmul).

Strategy (per core, batch-parallel over 8 cores):
  - Fold w_pre @ w_post into one 128x128 matrix per band on the host (no
    nonlinearity between the linears); biases applied host-side.
  - Carve the frequency axis into 33 aligned segments of 32 bins (grid
    f + 22 = 32*j + u). Every band spans <= 2 adjacent segments, so the
    folded weights form a block-TRIDIAGONAL [33x33] structure of 128x128
    blocks (97 nonzero blocks) over the feature layout g = c*32 + u.
  - Host passes x already cast to fp16, zero-padded to the segment grid,
    and permuted to [C, u(32), j(33), T] — exactly the SBUF layout
    [g = c*32+u, j*T + t], so each SBUF partition's data is ONE contiguous
    DRAM run (multi-KB DMA descriptors at near-peak per-engine rate).
    On-chip data flow is pure: HBM->SBUF loads, fp16 matmuls with fp32
    PSUM accumulation, PSUM->SBUF cast copies, SBUF->HBM stores in the
    same layout. Zero on-chip transposes, gather/scatter, or packing.
    Host permutes/casts the output back to [C, T, F] fp32.
  - DMA is the bottleneck (~20 MB/core, 16 DMA engines ~22 GB/s each):
    traffic is split into ~56 DMAs across engines/queues for concurrency,
    sized small at the pipeline head (fast start) and tail (fast drain).
"""

import numpy as np

import concourse.bass as bass
import concourse.tile as tile
from concourse import bacc, mybir
from concourse.bass_utils import run_bass_kernel_spmd


# ---- problem constants (hardcoded per spec) ----
B, C, T, F = 8, 4, 1000, 1025
N_CORES = 8
SEG = 32
FOFF = 22  # grid phase: f + FOFF = 32*j + u; band starts align for f >= 490
NSEG = (F - 1 + FOFF) // SEG + 1  # 33
FPAD = NSEG * SEG  # 1056 padded f rows
P = 128
T_CHUNKS = [(0, 512), (512, 488)]
LOAD_GROUPS = [2, 3, 4, 6, 8, 10]  # j-segments per load group (sum 33)
STORE_GROUPS = [8, 8, 8, 5, 2, 2]  # j-segments per store group (sum 33)

_F32 = mybir.dt.float32
_F16 = mybir.dt.float16


def _build_bands():
    f, interval = 0, 4
    groups = []
    while f < F:
        end = min(f + interval, F)
        groups.append((f, end))
        f = end
        if interval < 32:
            interval += 1
    return groups  # list of (start, end), disjoint, covering [0, F)


def _block_structure():
    """Nonzero (j_out, j_in) block pairs, grouped by j_out (ascending j_in)."""
    bands = _build_bands()
    pairs = set()
    for start, end in bands:
        segs = set(range((start + FOFF) // SEG, (end - 1 + FOFF) // SEG + 1))
        for ji in segs:
            for jo in segs:
                pairs.add((jo, ji))
    jin_lists = [sorted(ji for (jo, ji) in pairs if jo == j) for j in range(NSEG)]
    return bands, jin_lists


def _build_weight_blocks(w_pre, w_post):
    """Host: fold per-band linears and scatter into segment-pair blocks."""
    bands, jin_lists = _block_structure()
    wc = np.einsum(
        "kio,kod->kid", w_pre.astype(np.float64), w_post.astype(np.float64)
    )  # [45, 128, 128], both feature dims indexed by w*4 + c
    blocks = {}
    for k, (start, end) in enumerate(bands):
        fs = np.arange(start, end)
        js = (fs + FOFF) // SEG
        us = (fs + FOFF) % SEG
        for ji in np.unique(js):
            for jo in np.unique(js):
                key = (int(jo), int(ji))
                if key not in blocks:
                    blocks[key] = np.zeros((P, P), dtype=np.float64)
                blk = blocks[key]
                mi = js == ji
                mo = js == jo
                wi = fs[mi] - start
                wo = fs[mo] - start
                for ci in range(C):
                    for co in range(C):
                        blk[np.ix_(ci * SEG + us[mi], co * SEG + us[mo])] = wc[k][
                            np.ix_(wi * C + ci, wo * C + co)
                        ]
    order = [(jo, ji) for jo in range(NSEG) for ji in jin_lists[jo]]
    wall = np.stack([blocks[key] for key in order]).astype(np.float16)
    offs = np.cumsum([0] + [len(jl) for jl in jin_lists])
    return wall, jin_lists, offs


def _bias_field(bands, b_pre, w_post, b_post):
    """bias[c, f]: the constant added to out[., c, ., f]."""
    bc = (
        np.einsum("ko,kod->kd", b_pre.astype(np.float64), w_post.astype(np.float64))
        + b_post.astype(np.float64)
    )
    field = np.zeros((C, F), dtype=np.float64)
    for k, (start, end) in enumerate(bands):
        for c in range(C):
            field[c, start:end] = bc[k, (np.arange(end - start)) * C + c]
    return field.astype(np.float32)


def _build_nc(jin_lists, offs, nblk):
    nc = bacc.Bacc("TRN2", target_bir_lowering=False, debug=False)
    xs = nc.dram_tensor("xs", [C, SEG, NSEG, T], _F16, kind="ExternalInput")
    wall = nc.dram_tensor("wall", [P, nblk * P], _F16, kind="ExternalInput")
    ys = nc.dram_tensor("ys", [C, SEG, NSEG, T], _F16, kind="ExternalOutput")

    # wall load split points: small first ranges so early jo can start
    wall_splits = [0, 2, 6, 13, 25]
    while wall_splits[-1] < nblk:
        wall_splits.append(min(wall_splits[-1] + 18, nblk))

    with tile.TileContext(nc) as tc:
        import contextlib

        ctx = contextlib.ExitStack()
        with ctx:
            wall_pool = ctx.enter_context(tc.tile_pool(name="wall", bufs=1))
            at_pools = [
                ctx.enter_context(tc.tile_pool(name=f"atg{i}", bufs=1))
                for i in range(len(LOAD_GROUPS))
            ]
            y_pools = [
                ctx.enter_context(tc.tile_pool(name=f"yg{i}", bufs=1))
                for i in range(len(STORE_GROUPS))
            ]
            psy_pool = ctx.enter_context(
                tc.tile_pool(name="psy", bufs=6, space="PSUM")
            )

            # ---- resident fp16 folded weights [128, nblk*128] ----
            wall_sb = wall_pool.tile([P, nblk * P], _F16)
            for lo, hi in zip(wall_splits, wall_splits[1:]):
                nc.scalar.dma_start(
                    wall_sb[:, lo * P : hi * P], wall.ap()[:, lo * P : hi * P]
                )

            # ---- input loads: [g = c*32+u, j*T + t] per group ----
            at_tiles = []  # (j0, tile) per group
            j0 = 0
            for gi, gn in enumerate(LOAD_GROUPS):
                at_tiles.append((j0, at_pools[gi].tile([P, gn * T], _F16, name=f"atg{gi}")))
                j0 += gn

            load_engines = [nc.sync, nc.sync, nc.scalar, nc.gpsimd]
            for gi, gn in enumerate(LOAD_GROUPS):
                j0, at_t = at_tiles[gi]
                for c in range(C):
                    src = xs.ap()[c, :, j0 : j0 + gn, :]
                    dst = at_t[c * SEG : (c + 1) * SEG, :].rearrange(
                        "u (j t) -> u j t", j=gn
                    )
                    load_engines[c].dma_start(dst, src)

            def at_slice(ji, t0, tn):
                for gi, gn in enumerate(LOAD_GROUPS):
                    j0, at_t = at_tiles[gi]
                    if j0 <= ji < j0 + gn:
                        return at_t[:, (ji - j0) * T + t0 : (ji - j0) * T + t0 + tn]
                raise AssertionError(ji)

            # ---- y staging tiles per store group ----
            y_tiles = []
            j0 = 0
            for gi, gn in enumerate(STORE_GROUPS):
                y_tiles.append((j0, y_pools[gi].tile([P, gn * T], _F16, name=f"yg{gi}")))
                j0 += gn

            # ---- matmul wavefront over jo, PSUM -> y copies, group stores ----
            gi_store = 0
            for jo in range(NSEG):
                jins = jin_lists[jo]
                nw = len(jins)
                w0 = offs[jo]
                yj0, y_t = y_tiles[gi_store]
                for t0, tn in T_CHUNKS:
                    psy = psy_pool.tile([P, 512], _F32, name="psy")
                    for i, ji in enumerate(jins):
                        nc.tensor.matmul(
                            psy[:, 0:tn],
                            lhsT=wall_sb[:, (w0 + i) * P : (w0 + i + 1) * P],
                            rhs=at_slice(ji, t0, tn),
                            start=(i == 0),
                            stop=(i == nw - 1),
                        )
                    dst = y_t[:, (jo - yj0) * T + t0 : (jo - yj0) * T + t0 + tn]
                    if jo % 2 == 0:
                        nc.scalar.copy(dst, psy[:, 0:tn])
                    else:
                        nc.vector.tensor_copy(dst, psy[:, 0:tn])

                # group finished -> store it
                gn = STORE_GROUPS[gi_store]
                if jo == yj0 + gn - 1:
                    last = gi_store == len(STORE_GROUPS) - 1
                    # split the final group by u-halves for a short drain tail
                    usplits = [(0, SEG // 2), (SEG // 2, SEG // 2)] if last else [
                        (0, SEG)
                    ]
                    store_engines = [nc.sync, nc.sync, nc.gpsimd, nc.gpsimd]
                    for u0, un in usplits:
                        for c in range(C):
                            dst = ys.ap()[c, u0 : u0 + un, yj0 : yj0 + gn, :]
                            src = y_t[
                                c * SEG + u0 : c * SEG + u0 + un, :
                            ].rearrange("u (j t) -> u j t", j=gn)
                            store_engines[c].dma_start(dst, src)
                    gi_store += 1
    nc.compile()
    return nc


_CACHE = {}


def build_in_maps(x, wall):
    """Host prep: weights to [g_in, blk*128+g_out]; x cast fp16, padded to
    the 1056-row segment grid, permuted to [C, u(32), j(33), T] so that each
    SBUF partition g = c*32+u reads one contiguous DRAM run."""
    nblk = wall.shape[0]
    wall2 = np.ascontiguousarray(wall.transpose(1, 0, 2).reshape(P, nblk * P))
    xp = np.zeros((B, C, FPAD, T), dtype=np.float16)
    xp[:, :, FOFF : FOFF + F, :] = np.asarray(x, np.float32).astype(
        np.float16
    ).transpose(0, 1, 3, 2)
    xp = np.ascontiguousarray(
        xp.reshape(B, C, NSEG, SEG, T).transpose(0, 1, 3, 2, 4)
    )  # [B, C, u, j, T]
    return [{"xs": xp[b], "wall": wall2} for b in range(N_CORES)]


def kernel(x, w_pre, b_pre, w_post, b_post):
    x = np.asarray(x, dtype=np.float32)
    w_pre = np.asarray(w_pre, dtype=np.float32)
    b_pre = np.asarray(b_pre, dtype=np.float32)
    w_post = np.asarray(w_post, dtype=np.float32)
    b_post = np.asarray(b_post, dtype=np.float32)

    bands, _ = _block_structure()
    wall, jin_lists, offs = _build_weight_blocks(w_pre, w_post)
    nblk = wall.shape[0]

    if "nc" not in _CACHE:
        _CACHE["nc"] = _build_nc(jin_lists, offs, nblk)
    nc = _CACHE["nc"]

    in_maps = build_in_maps(x, wall)
    res = run_bass_kernel_spmd(nc, in_maps, core_ids=list(range(N_CORES)))
    yp = np.stack([res.results[b]["ys"] for b in range(N_CORES)])  # [B,C,u,j,T]
    out = (
        yp.transpose(0, 1, 4, 3, 2)  # [B, C, T, j, u]
        .reshape(B, C, T, FPAD)[:, :, :, FOFF : FOFF + F]
        .astype(np.float32)
    )

    if np.any(b_pre) or np.any(b_post):
        field = _bias_field(bands, b_pre, w_post, b_post)
        out = out + field[None, :, None, :]
    return np.ascontiguousarray(out)
